# revision 13
# baseline (speedup 1.0000x reference)
"""Trainium2 Bass kernel for MesoNet-style 3-layer NNConv GNN (8 NeuronCores).

Strategy:
  - Edges are sharded across 8 cores BY DESTINATION node (host-side sort), so the
    scatter-mean is core-local. Node features for each layer are exchanged with an
    AllGather (each core owns a contiguous 2048-node slice).
  - Per-edge weight matrices are never materialized. Using
        msg_e = sum_k h[e,k] * (Xsrc @ T_k),  T_k[i,o] = l2w[k, i*128+o]
    the per-edge matmul becomes 33 dense [E,128]@[128,128] matmuls on the tensor
    engine (k-slot 32 carries the l2 bias) + a per-edge weighted combine on the
    vector engine.
  - Scatter-mean is a matmul with host-built block one-hot matrices P (values
    1/deg), exact within fp32.
  - All matmuls run as float32r (full-rate fp32 PE mode).
"""

import os
import numpy as np

N = 16384          # nodes
E = 32768          # edges
D = 128            # feature dim
EDGE_DIM = 10
EH = 32            # edge hidden
KS = EH + 1        # k-slots incl. l2-bias slot
C = 8              # cores
NL = N // C        # nodes per core
NT = NL // 128     # node tiles per core (16)

_LAST_RESULTS = None  # BassKernelResults of the most recent hw run (for test.py)


# --------------------------------------------------------------------------
# Host-side preparation: shard edges by dst, sort, pad, build P blocks.
# --------------------------------------------------------------------------

def _prepare(x, edge_index, edge_attr,
             w1_l1, b1_l1, w1_l2, b1_l2, w1_root, b1,
             w2_l1, b2_l1, w2_l2, b2_l2, w2_root, b2):
    src = np.asarray(edge_index[0], dtype=np.int64)
    dst = np.asarray(edge_index[1], dtype=np.int64)
    x = np.asarray(x, dtype=np.float32)
    edge_attr = np.asarray(edge_attr, dtype=np.float32)

    deg = np.bincount(dst, minlength=N).astype(np.float32)
    recip = 1.0 / np.maximum(deg, 1.0)          # [N]

    core_of = dst // NL
    order = np.lexsort((dst, core_of))          # sort by (core, dst)
    src_s, dst_s = src[order], dst[order]
    ea_s = edge_attr[order]
    core_s = core_of[order]

    counts = np.bincount(core_s, minlength=C)
    EB = int(np.max(np.ceil(counts / 128)))     # e-blocks per core (uniform)
    E_pad = EB * 128

    per_core = []
    bounds = np.concatenate([[0], np.cumsum(counts)])
    tri_lists = []
    for c in range(C):
        lo, hi = bounds[c], bounds[c + 1]
        ne = hi - lo
        srcp = np.full(E_pad, N, dtype=np.int32)          # N -> zero row
        srcp[:ne] = src_s[lo:hi]
        dstl = np.full(E_pad, -1, dtype=np.int64)         # local dst, -1 = pad
        dstl[:ne] = dst_s[lo:hi] - c * NL
        eaT = np.zeros((EDGE_DIM + 1, E_pad), dtype=np.float32)
        eaT[:EDGE_DIM, :ne] = ea_s[lo:hi].T
        eaT[EDGE_DIM, :ne] = 1.0                          # l1-bias row

        # per-core (e-block, n-tile) -> P data [128,128]
        tris = {}
        rec_l = recip[c * NL:(c + 1) * NL]
        for b in range(EB):
            dblk = dstl[b * 128:(b + 1) * 128]
            valid = dblk >= 0
            if not valid.any():
                continue
            for nt in np.unique(dblk[valid] // 128):
                nt = int(nt)
                P = np.zeros((128, 128), dtype=np.float32)
                sel = valid & (dblk // 128 == nt)
                j = np.nonzero(sel)[0]
                m = (dblk[j] - nt * 128).astype(np.int64)
                P[j, m] = rec_l[dblk[j]]
                tris[(b, nt)] = P
        tri_lists.append(tris)
        per_core.append(dict(srcp=srcp, eaT=eaT))

    # SPMD: the triple structure is baked into the (shared) program, so use
    # the union over cores; cores contribute zero-P (no-op) where unused.
    union = sorted(set().union(*[set(t.keys()) for t in tri_lists]))
    T_UNI = len(union)
    tri_meta = [(b, nt) for (b, nt) in union]
    zeros = np.zeros((128, 128), dtype=np.float32)
    for c in range(C):
        Pmat = np.concatenate(
            [tri_lists[c].get(key, zeros) for key in union], axis=1)
        per_core[c].update(Pmat=Pmat)

    # gather index layout: [128, EB] int32, col b = indices of block b
    for c in range(C):
        per_core[c]["gidx"] = np.ascontiguousarray(
            per_core[c]["srcp"].reshape(EB, 128).T).astype(np.int32)
        del per_core[c]["srcp"]

    def l1_aug(w, b):
        a = np.zeros((EDGE_DIM + 1, KS), dtype=np.float32)
        a[:EDGE_DIM, :EH] = w
        a[EDGE_DIM, :EH] = b
        a[EDGE_DIM, EH] = 1.0
        return a

    def t_aug(l2w, l2b):
        t = np.zeros((D, KS * 128), dtype=np.float32)
        w = np.asarray(l2w, np.float32).reshape(EH, D, D)     # [k, i, o]
        t[:, :EH * 128] = w.transpose(1, 0, 2).reshape(D, EH * 128)
        t[:, EH * 128:] = np.asarray(l2b, np.float32).reshape(D, D)
        return t

    shared = dict(
        x_full=np.concatenate([x, np.zeros((1, D), np.float32)], axis=0),
        l1w1=l1_aug(w1_l1, b1_l1), l1w2=l1_aug(w2_l1, b2_l1),
        T1=t_aug(w1_l2, b1_l2), T2=t_aug(w2_l2, b2_l2),
        root1=np.asarray(w1_root, np.float32), root2=np.asarray(w2_root, np.float32),
        biasbc1=np.broadcast_to(np.asarray(b1, np.float32), (128, D)).copy(),
        biasbc2=np.broadcast_to(np.asarray(b2, np.float32), (128, D)).copy(),
    )
    for c in range(C):
        per_core[c]["x_locT"] = np.ascontiguousarray(x[c * NL:(c + 1) * NL].T)

    return dict(EB=EB, E_pad=E_pad, T_UNI=T_UNI, tri_meta=tri_meta,
                shared=shared, per_core=per_core)


# --------------------------------------------------------------------------
# Numpy emulation of the sharded algorithm (validates host prep + math).
# --------------------------------------------------------------------------

def kernel_numpy(**inputs):
    prep = _prepare(**inputs)
    EB, T_UNI = prep["EB"], prep["T_UNI"]
    sh = prep["shared"]
    h_full = sh["x_full"].copy()                 # [N+1, 128], last row zero

    def layer(h_full, l1w, T, root, biasbc, relu, h_locT_all):
        new_full = np.zeros((N + 1, D), np.float32)
        for c in range(C):
            pc = prep["per_core"][c]
            eaT, gidx, Pmat = pc["eaT"], pc["gidx"], pc["Pmat"]
            h = np.maximum(eaT.T @ l1w, 0.0)     # [E_pad, 33]
            agg = np.zeros((NT, 128, D), np.float32)
            for b in range(EB):
                xg = h_full[gidx[:, b]]          # [128, 128]
                G = xg @ T                        # [128, 33*128]
                msg = np.zeros((128, D), np.float32)
                for k in range(KS):
                    msg += h[b * 128:(b + 1) * 128, k:k + 1] * G[:, k * 128:(k + 1) * 128]
                for t, (tb, nt) in enumerate(prep["tri_meta"]):
                    if tb == b:
                        P = Pmat[:, t * 128:(t + 1) * 128]
                        agg[nt] += P.T @ msg
            hl = h_locT_all[c]                   # [128 feat, 2048]
            for nt in range(NT):
                out = hl[:, nt * 128:(nt + 1) * 128].T @ root + agg[nt] + biasbc[:, :]
                if relu:
                    out = np.maximum(out, 0.0)
                new_full[c * NL + nt * 128: c * NL + (nt + 1) * 128] = out
        new_locT = [np.ascontiguousarray(new_full[c * NL:(c + 1) * NL].T)
                    for c in range(C)]
        return new_full, new_locT

    x_locT = [prep["per_core"][c]["x_locT"] for c in range(C)]
    h1, h1T = layer(h_full, sh["l1w1"], sh["T1"], sh["root1"], sh["biasbc1"], True, x_locT)
    h2, h2T = layer(h1, sh["l1w2"], sh["T2"], sh["root2"], sh["biasbc2"], True, h1T)
    h3, _ = layer(h2, sh["l1w2"], sh["T2"], sh["root2"], sh["biasbc2"], False, h2T)
    return h3[:N]


# --------------------------------------------------------------------------
# Bass program.
# --------------------------------------------------------------------------

def _build(prep):
    import concourse.bacc as bacc
    import concourse.bass as bass
    import concourse.tile as tile
    import concourse.mybir as mybir
    from concourse.masks import make_identity

    EB, E_pad, T_UNI = prep["EB"], prep["E_pad"], prep["T_UNI"]
    f32 = mybir.dt.float32
    f32r = mybir.dt.float32r
    i32 = mybir.dt.int32

    nc = bacc.Bacc("TRN2", target_bir_lowering=False, debug=False,
                   num_devices=C)

    # ---- I/O ----
    ein = {}
    def inp(name, shape, dtype=f32):
        ein[name] = nc.dram_tensor(name, list(shape), dtype, kind="ExternalInput")
        return ein[name]

    x_full = inp("x_full", (N + 1, D))
    eaT_d = inp("eaT", (EDGE_DIM + 1, E_pad))
    gidx_d = inp("gidx", (128, EB), i32)
    Pmat_d = inp("Pmat", (128, T_UNI * 128))
    xlocT_d = inp("x_locT", (128, NL))
    l1w1_d = inp("l1w1", (EDGE_DIM + 1, KS))
    l1w2_d = inp("l1w2", (EDGE_DIM + 1, KS))
    T1_d = inp("T1", (D, KS * 128))
    T2_d = inp("T2", (D, KS * 128))
    root1_d = inp("root1", (D, D))
    root2_d = inp("root2", (D, D))
    bb1_d = inp("biasbc1", (128, D))
    bb2_d = inp("biasbc2", (128, D))
    out_d = nc.dram_tensor("out", [NL, D], f32, kind="ExternalOutput")

    # internal DRAM
    agb = [nc.dram_tensor(f"agb{i}", [NL, D], f32) for i in range(2)]
    hf = [nc.dram_tensor(f"hf{i}", [N + 1, D], f32, addr_space="Shared")
          for i in range(2)]

    RG = [list(range(C))]

    with tile.TileContext(nc) as tc:
        with (
            tc.tile_pool(name="const", bufs=1) as cp,
            tc.tile_pool(name="work", bufs=3) as wp,
            tc.tile_pool(name="gp", bufs=2, space="PSUM") as gp,
            tc.tile_pool(name="scr", bufs=2, space="PSUM") as sp,
            tc.tile_pool(name="aggp", bufs=1, space="PSUM") as ap_,
        ):
            # ---- persistent SBUF ----
            def load(dram, shape, dtype=f32, tag=None):
                t = cp.tile(list(shape), dtype, tag=tag or dram.name)
                nc.sync.dma_start(out=t[:], in_=dram[:, :])
                return t

            # float32r matmul operands must be rounded by a compute engine op;
            # DMA loads are staged through a scratch tile + gpsimd copy.
            def load_r(dram, shape, tag):
                t = cp.tile(list(shape), f32r, tag=tag)
                CH = 4096
                for j0 in range(0, shape[1], CH):
                    w = min(CH, shape[1] - j0)
                    ldscr = wp.tile([shape[0], CH], f32, tag="ldscr", name="ldscr")
                    nc.sync.dma_start(out=ldscr[:, :w],
                                      in_=dram[:, j0:j0 + w])
                    nc.gpsimd.tensor_copy(out=t[:, j0:j0 + w], in_=ldscr[:, :w])
                return t

            T1s = load_r(T1_d, (D, KS * 128), "T1s")
            T2s = load_r(T2_d, (D, KS * 128), "T2s")
            Ps = load_r(Pmat_d, (128, T_UNI * 128), "Ps")
            root1s = load_r(root1_d, (D, D), "root1s")
            root2s = load_r(root2_d, (D, D), "root2s")
            xlocTs = load_r(xlocT_d, (128, NL), "xlocTs")
            eaTs = load(eaT_d, (EDGE_DIM + 1, E_pad))
            gidxs = load(gidx_d, (128, EB), i32)
            l1w1s = load(l1w1_d, (EDGE_DIM + 1, KS))
            l1w2s = load(l1w2_d, (EDGE_DIM + 1, KS))
            bb1s = load(bb1_d, (128, D))
            bb2s = load(bb2_d, (128, D))
            hlocT1 = cp.tile([128, NL], f32r, tag="hlocT1")
            hlocT2 = cp.tile([128, NL], f32r, tag="hlocT2")
            h1s = cp.tile([128, EB * KS], f32, tag="h1s")
            h2s = cp.tile([128, EB * KS], f32, tag="h2s")
            ident = cp.tile([128, 128], f32, tag="ident")
            make_identity(nc, ident[:])
            zrow = cp.tile([1, D], f32, tag="zrow")
            nc.vector.memset(zrow[:], 0.0)
            for i in range(2):
                nc.sync.dma_start(out=hf[i][N:N + 1, :], in_=zrow[:])

            # ---- edge MLP h (both layer types, upfront) ----
            for l1ws, hs in ((l1w1s, h1s), (l1w2s, h2s)):
                for b in range(EB):
                    hp = sp.tile([128, KS], f32, tag="scr")
                    nc.tensor.matmul(
                        out=hp[:], lhsT=eaTs[:, b * 128:(b + 1) * 128],
                        rhs=l1ws[:], start=True, stop=True)
                    nc.scalar.activation(
                        out=hs[:, b * KS:(b + 1) * KS], in_=hp[:],
                        func=mybir.ActivationFunctionType.Relu)

            def emit_layer(gsrc, hs, Ts, roots, bbs, relu, hlocT_in, hlocT_out,
                           out_rows):
                pc_meta = prep["tri_meta"]
                agg = [ap_.tile([128, 512], f32, tag=f"agg{g}", name=f"agg{g}")
                       for g in range(4)]

                def aslice(nt):
                    return agg[nt // 4][:, (nt % 4) * 128:((nt % 4) + 1) * 128]

                # PSUM accumulation flags are zero-region (bank) granular:
                # start=True only on the first matmul into each [128,512] bank,
                # stop=True only on the last one.
                seq = [("root", nt, nt) for nt in range(NT)]
                seq += [("tri", t, nt) for t, (tb, nt) in enumerate(pc_meta)]
                last_in_bank = {}
                for i, (_, _, nt) in enumerate(seq):
                    last_in_bank[nt // 4] = i
                root_stop = {}
                tri_stop = {}
                for i, (kind, idx, nt) in enumerate(seq):
                    is_stop = last_in_bank[nt // 4] == i
                    (root_stop if kind == "root" else tri_stop)[idx] = is_stop

                # root term first: opens each bank's accumulation group
                for nt in range(NT):
                    nc.tensor.matmul(
                        out=aslice(nt),
                        lhsT=hlocT_in[:, nt * 128:(nt + 1) * 128],
                        rhs=roots[:],
                        start=(nt % 4 == 0), stop=root_stop[nt])

                tri_by_b = {}
                for t, (tb, nt) in enumerate(pc_meta):
                    tri_by_b.setdefault(tb, []).append((t, nt, tri_stop[t]))

                for b in range(EB):
                    xg = wp.tile([128, 128], f32, tag="xg")
                    nc.gpsimd.indirect_dma_start(
                        out=xg[:], out_offset=None, in_=gsrc[:, :],
                        in_offset=bass.IndirectOffsetOnAxis(
                            ap=gidxs[:, b:b + 1], axis=0))
                    tp = sp.tile([128, 128], f32, tag="scr")
                    nc.tensor.transpose(out=tp[:], in_=xg[:], identity=ident[:])
                    xsT = wp.tile([128, 128], f32r, tag="xsT")
                    nc.scalar.activation(
                        out=xsT[:], in_=tp[:],
                        func=mybir.ActivationFunctionType.Copy)
                    msg = wp.tile([128, 128], f32r, tag="msg")
                    nk = 0
                    for kg in range((KS + 3) // 4):
                        k0 = kg * 4
                        kn = min(4, KS - k0)
                        G = gp.tile([128, 512], f32, tag="G")
                        nc.tensor.matmul(
                            out=G[:, :kn * 128],
                            lhsT=xsT[:],
                            rhs=Ts[:, k0 * 128:(k0 + kn) * 128],
                            start=True, stop=True)
                        for j in range(kn):
                            k = k0 + j
                            scal = (1.0 if k == EH
                                    else hs[:, b * KS + k:b * KS + k + 1])
                            gsl = G[:, j * 128:(j + 1) * 128]
                            if nk == 0:
                                nc.vector.tensor_scalar_mul(
                                    out=msg[:], in0=gsl, scalar1=scal)
                            else:
                                nc.vector.scalar_tensor_tensor(
                                    out=msg[:], in0=gsl, scalar=scal, in1=msg[:],
                                    op0=mybir.AluOpType.mult,
                                    op1=mybir.AluOpType.add)
                            nk += 1
                    for (t, nt, stop) in tri_by_b.get(b, ()):
                        nc.tensor.matmul(
                            out=aslice(nt),
                            lhsT=Ps[:, t * 128:(t + 1) * 128],
                            rhs=msg[:],
                            start=False, stop=stop)

                for nt in range(NT):
                    nh = wp.tile([128, 128], f32, tag="nh")
                    nc.vector.scalar_tensor_tensor(
                        out=nh[:], in0=aslice(nt), scalar=1.0, in1=bbs[:],
                        op0=mybir.AluOpType.mult, op1=mybir.AluOpType.add)
                    if relu:
                        nh2 = wp.tile([128, 128], f32, tag="nh2")
                        nc.scalar.activation(
                            out=nh2[:], in_=nh[:],
                            func=mybir.ActivationFunctionType.Relu)
                        nh = nh2
                    nc.sync.dma_start(
                        out=out_rows[nt * 128:(nt + 1) * 128, :], in_=nh[:])
                    if hlocT_out is not None:
                        tp2 = sp.tile([128, 128], f32, tag="scr")
                        nc.tensor.transpose(out=tp2[:], in_=nh[:],
                                            identity=ident[:])
                        nc.scalar.activation(
                            out=hlocT_out[:, nt * 128:(nt + 1) * 128], in_=tp2[:],
                            func=mybir.ActivationFunctionType.Copy)

            # layer 1
            emit_layer(x_full, h1s, T1s, root1s, bb1s, True, xlocTs, hlocT1,
                       agb[0])
            nc.gpsimd.collective_compute(
                "AllGather", mybir.AluOpType.bypass, replica_groups=RG,
                ins=[agb[0][:, :].opt()], outs=[hf[0][0:N, :].opt()])
            # layer 2
            emit_layer(hf[0], h2s, T2s, root2s, bb2s, True, hlocT1, hlocT2,
                       agb[1])
            nc.gpsimd.collective_compute(
                "AllGather", mybir.AluOpType.bypass, replica_groups=RG,
                ins=[agb[1][:, :].opt()], outs=[hf[1][0:N, :].opt()])
            # layer 3
            emit_layer(hf[1], h2s, T2s, root2s, bb2s, False, hlocT2, None,
                       out_d)

    nc.compile()
    return nc


def _in_maps(prep):
    sh = prep["shared"]
    maps = []
    for c in range(C):
        pc = prep["per_core"][c]
        maps.append(dict(
            x_full=sh["x_full"], eaT=pc["eaT"], gidx=pc["gidx"],
            Pmat=pc["Pmat"].astype(np.float32), x_locT=pc["x_locT"],
            l1w1=sh["l1w1"], l1w2=sh["l1w2"], T1=sh["T1"], T2=sh["T2"],
            root1=sh["root1"], root2=sh["root2"],
            biasbc1=sh["biasbc1"], biasbc2=sh["biasbc2"],
        ))
    return maps


def kernel(**inputs):
    global _LAST_RESULTS
    prep = _prepare(**inputs)
    nc = _build(prep)
    maps = _in_maps(prep)

    if os.environ.get("BASS_GNN_SIM"):
        from concourse.bass_interp import MultiCoreSim
        sim = MultiCoreSim(nc, C)
        for c in range(C):
            for k, v in maps[c].items():
                sim.cores[c].tensor(k)[:] = v
        sim.simulate(check_with_hw=False)
        outs = [np.array(sim.cores[c].mem_tensor("out")) for c in range(C)]
    else:
        from concourse.bass_utils import run_bass_kernel_spmd
        res = run_bass_kernel_spmd(
            nc, maps, list(range(C)),
            trace=bool(os.environ.get("BASS_GNN_TRACE")))
        _LAST_RESULTS = res
        outs = [res.results[c]["out"] for c in range(C)]

    return np.concatenate(outs, axis=0)
